# revision 3
# baseline (speedup 1.0000x reference)
"""Trainium2 Bass kernel for nn_Attention (B=2, L=2048, DIM=1024, H=16, D=64).

Sharding: 8 cores, each handles one (b, 4-head-group) pair — data parallel
on B (cores 0-3 -> b=0, cores 4-7 -> b=1), tensor parallel on heads
(4 heads per core). The output projection is computed per-core over the
core's 4 heads; the host sums the 4 partials per batch and adds the bias.

Device-side layout (per core, everything transposed so the contraction dim
sits on SBUF partitions):
  xT   [DIM, L]     x[b]^T
  wqk  [DIM, 512]   [Wq_scaled | Wk]^T for the core's 4 heads (Wq pre-scaled
                    by qk_scale * s * log(L) so exp() needs no extra scale)
  wv   [DIM, 256]   Wv^T for the 4 heads
  wp   [256, DIM]   proj_w[:, head_slice]^T
  y    [L, DIM]     per-core partial output (pre-bias)

Attention per head: S^T tiles [m,l] = (K Q^T) via matmul(lhsT=K^T, rhs=Q^T);
exp on ScalarE; A·V via matmul(lhsT=[V | ones], rhs=P^T) which also yields
the softmax denominators in the extra output row; normalize with
reciprocal + gpsimd partition_broadcast.
"""

import math
import sys

sys.path.insert(0, "/opt/trn_rl_repo")

import ml_dtypes
import numpy as np

import concourse.bass as bass  # noqa: F401  (bass types used via tile/bacc)
import concourse.tile as tile
from concourse import bacc, bass_utils, mybir

B, L, DIM, H, D = 2, 2048, 1024, 16, 64
N_CORES = 8
HL = 4  # heads per core
F = HL * D  # 256: per-core head feature width
LC, LT, CT = 512, 128, 128  # l-chunk, l/m-tile, contraction tile
N_LC, N_LT, N_CT = L // LC, L // LT, DIM // CT

DT = mybir.dt.bfloat16
NP_DT = ml_dtypes.bfloat16
F32 = mybir.dt.float32

_build_cache = {}


def _build(with_mask: bool):
    if with_mask in _build_cache:
        return _build_cache[with_mask]

    nc = bacc.Bacc("TRN2", target_bir_lowering=False, debug=False, num_devices=N_CORES)
    xT = nc.dram_tensor("xT", [DIM, L], DT, kind="ExternalInput").ap()
    wqk = nc.dram_tensor("wqk", [DIM, 2 * F], DT, kind="ExternalInput").ap()
    wv = nc.dram_tensor("wv", [DIM, F], DT, kind="ExternalInput").ap()
    wp = nc.dram_tensor("wp", [F, DIM], DT, kind="ExternalInput").ap()
    if with_mask:
        maskT = nc.dram_tensor("maskT", [HL, L, L], F32, kind="ExternalInput").ap()
    y = nc.dram_tensor("y", [L, DIM], F32, kind="ExternalOutput").ap()

    Exp = mybir.ActivationFunctionType.Exp

    with tile.TileContext(nc) as tc:
        with (
            tc.tile_pool(name="consts", bufs=1) as consts,
            tc.tile_pool(name="work", bufs=4) as work,
            tc.tile_pool(name="outb", bufs=4) as outb,
            tc.tile_pool(name="drp", bufs=4, space="DRAM") as drp,
            tc.tile_pool(name="ps_mm", bufs=4, space="PSUM") as ps_mm,
            tc.tile_pool(name="ps_acc", bufs=4, space="PSUM") as ps_acc,
        ):
            # ---- load inputs ----
            xT_sb = consts.tile([128, N_CT, L], DT)
            for c in range(N_CT):
                nc.sync.dma_start(out=xT_sb[:, c, :], in_=xT[c * 128 : (c + 1) * 128, :])
            wqk_sb = consts.tile([128, N_CT, 2 * F], DT)
            for c in range(N_CT):
                nc.sync.dma_start(out=wqk_sb[:, c, :], in_=wqk[c * 128 : (c + 1) * 128, :])
            wv_sb = consts.tile([128, N_CT, F], DT)
            for c in range(N_CT):
                nc.sync.dma_start(out=wv_sb[:, c, :], in_=wv[c * 128 : (c + 1) * 128, :])
            wp_sb = consts.tile([128, 2, DIM], DT)
            for t in range(2):
                nc.sync.dma_start(out=wp_sb[:, t, :], in_=wp[t * 128 : (t + 1) * 128, :])

            # ---- stage A: Q^T/K^T [f, l] (f = [q 4 heads | k 4 heads] * 64) ----
            qkT_sb = consts.tile([128, 4, L], DT)
            for ft in range(4):
                for lc in range(N_LC):
                    ps = ps_mm.tile([128, LC], F32, name="ps_qk", tag="mm")
                    for c in range(N_CT):
                        nc.tensor.matmul(
                            ps,
                            lhsT=wqk_sb[:, c, ft * 128 : (ft + 1) * 128],
                            rhs=xT_sb[:, c, lc * LC : (lc + 1) * LC],
                            start=(c == 0),
                            stop=(c == N_CT - 1),
                        )
                    nc.vector.tensor_copy(qkT_sb[:, ft, lc * LC : (lc + 1) * LC], ps)

            # ---- stage A2: V [m, (head, d)] + ones column ----
            v_sb = consts.tile([128, N_LT, HL, D + 1], DT)
            nc.vector.memset(v_sb[:, :, :, D : D + 1], 1.0)
            for lt in range(N_LT):
                ps = ps_mm.tile([128, F], F32, name="ps_v", tag="mm")
                for c in range(N_CT):
                    nc.tensor.matmul(
                        ps,
                        lhsT=xT_sb[:, c, lt * 128 : (lt + 1) * 128],
                        rhs=wv_sb[:, c, :],
                        start=(c == 0),
                        stop=(c == N_CT - 1),
                    )
                nc.vector.tensor_copy(
                    v_sb[:, lt, :, 0:D], ps.rearrange("p (h d) -> p h d", h=HL)
                )

            # ---- stage B: attention per head-pair ----
            oT_sb = consts.tile([128, 2, L], DT)
            for hp in range(2):  # head pairs (2*hp, 2*hp+1)
                for lc in range(N_LC):
                    lsl = slice(lc * LC, (lc + 1) * LC)
                    po = [
                        ps_acc.tile([128, LC], F32, name="po", tag="acc")
                        for _ in range(2)
                    ]
                    for mt in range(N_LT):
                        for hh in range(2):
                            h = 2 * hp + hh
                            off = 64 * hh
                            ps_s = ps_mm.tile([128, LC], F32, name="ps_s", tag="mm")
                            nc.tensor.matmul(
                                ps_s,
                                lhsT=qkT_sb[off : off + 64, 2 + hp, mt * 128 : (mt + 1) * 128],
                                rhs=qkT_sb[off : off + 64, hp, lsl],
                                start=True,
                                stop=True,
                            )
                            if with_mask:
                                mk = work.tile([128, LC], F32, name="mk", tag="mk")
                                nc.sync.dma_start(
                                    out=mk, in_=maskT[h, mt * 128 : (mt + 1) * 128, lsl]
                                )
                                nc.vector.tensor_add(ps_s, ps_s, mk)
                            pt = work.tile([128, LC], DT, name="pt", tag="pt")
                            nc.scalar.activation(pt, ps_s, Exp)
                            nc.tensor.matmul(
                                po[hh][0 : D + 1, :],
                                lhsT=v_sb[:, mt, h, :],
                                rhs=pt,
                                start=(mt == 0),
                                stop=(mt == N_LT - 1),
                            )
                    for hh in range(2):
                        off = 64 * hh
                        rr = work.tile([128, LC], F32, name="rr", tag="rr")
                        nc.vector.reciprocal(rr[D : D + 1, :], po[hh][D : D + 1, :])
                        # broadcast the reciprocal row to 64 partitions via a
                        # DRAM round-trip (step-0 partition APs are DRAM-only)
                        drow = drp.tile([1, LC], F32, name="drow", tag="dr")
                        nc.sync.dma_start(out=drow, in_=rr[D : D + 1, :])
                        rb = work.tile([64, LC], F32, name="rb", tag="rb")
                        bcast = bass.AP(
                            tensor=drow.tensor,
                            offset=drow.offset,
                            ap=[[0, 64]] + list(drow.ap[1:]),
                        )
                        nc.sync.dma_start(out=rb, in_=bcast)
                        nc.vector.tensor_mul(
                            oT_sb[off : off + 64, hp, lsl], po[hh][0:D, :], rb
                        )

            # ---- stage C: output projection (partial, pre-bias) ----
            for lt in range(N_LT):
                for oc in range(2):
                    osl = slice(oc * 512, (oc + 1) * 512)
                    ps = ps_mm.tile([128, 512], F32, name="ps_y", tag="mm")
                    for ft in range(2):
                        nc.tensor.matmul(
                            ps,
                            lhsT=oT_sb[:, ft, lt * 128 : (lt + 1) * 128],
                            rhs=wp_sb[:, ft, osl],
                            start=(ft == 0),
                            stop=(ft == 1),
                        )
                    yb = outb.tile([128, 512], F32, name="yb", tag="yb")
                    nc.vector.tensor_copy(yb, ps)
                    nc.sync.dma_start(out=y[lt * 128 : (lt + 1) * 128, osl], in_=yb)

    nc.compile()
    _build_cache[with_mask] = nc
    return nc


def _prepare_in_maps(x, attn_mask, qkv_w, proj_w, s, with_mask):
    qk_scale = D ** -0.5
    q_scale = qk_scale * float(s) * math.log(L)
    x = np.asarray(x, np.float32)
    qkv_w = np.asarray(qkv_w, np.float32)
    proj_w = np.asarray(proj_w, np.float32)

    in_maps = []
    for core in range(N_CORES):
        b = core // (N_CORES // B)
        h0 = (core % (N_CORES // B)) * HL
        fs = slice(h0 * D, h0 * D + F)
        wq = qkv_w[0 * DIM : 1 * DIM][fs] * q_scale  # [F, DIM]
        wk = qkv_w[1 * DIM : 2 * DIM][fs]
        wvm = qkv_w[2 * DIM : 3 * DIM][fs]
        m = {
            "xT": np.ascontiguousarray(x[b].T).astype(NP_DT),
            "wqk": np.ascontiguousarray(
                np.concatenate([wq, wk], axis=0).T
            ).astype(NP_DT),
            "wv": np.ascontiguousarray(wvm.T).astype(NP_DT),
            "wp": np.ascontiguousarray(proj_w[:, fs].T).astype(NP_DT),
        }
        if with_mask:
            m["maskT"] = np.ascontiguousarray(
                np.transpose(attn_mask[b, h0 : h0 + HL], (0, 2, 1))
            ).astype(np.float32)
        in_maps.append(m)
    return in_maps


def _postprocess(results, proj_b):
    gpb = N_CORES // B
    y = np.zeros((B, L, DIM), np.float32)
    for core in range(N_CORES):
        y[core // gpb] += results[core]["y"]
    y += np.asarray(proj_b, np.float32)[None, None, :]
    return y


def run(x, attn_mask, qkv_w, proj_w, proj_b, s, **spmd_kwargs):
    with_mask = bool(np.any(attn_mask))
    nc = _build(with_mask)
    in_maps = _prepare_in_maps(x, attn_mask, qkv_w, proj_w, s, with_mask)
    res = bass_utils.run_bass_kernel_spmd(
        nc, in_maps, core_ids=list(range(N_CORES)), **spmd_kwargs
    )
    return _postprocess(res.results, proj_b), res


def kernel(x, attn_mask, qkv_w, proj_w, proj_b, s):
    y, _ = run(x, attn_mask, qkv_w, proj_w, proj_b, s)
    return y
